# revision 62
# baseline (speedup 1.0000x reference)
"""Distributed attention kernel for Trainium2 (8 NeuronCores).

Problem: non-causal multi-head attention with GQA (16 q heads, 4 kv heads,
head_dim 64, dim 1024, batch 2, seqlen 2048), fp32.

Sharding (per the batch+head hint): core c in 0..7 handles batch b = c//4
and kv-head-group g = c%4 (q heads 4g..4g+3, kv head g). Each core holds the
full sequence, so softmax needs no communication. The output projection is
row-parallel: core (b, g) computes the partial product
O_g @ wo[256g:256(g+1), :] and the host sums the 4 partials per batch
(the gather/unshard step).

Per-core dataflow (v2 — exp-bound pipeline, PV in q-major orientation):
  xT = x[b].T                               (1024, S) fed from host
  QT = wq_g.T @ xT                          (256, S)  [head pair ft: rows
                                              0-63 = head 2ft, 64-127 = 2ft+1]
  KVT = [wk_g | wv_g].T @ xT                (128, S)  [K^T | V^T stacked]
  K^T duplicated to partitions 64-127 (swap-permutation identity matmul,
  1 cycle/row) so the two heads of a pair run as row-group-tiled matmuls
  on the PE array.
  V transposed (PE) to seq-major and packed as V' = [V | 1] (65 cols): the
  ones column makes the PV matmul produce the softmax denominator in place.
  Per (head pair ft, q-chunk 512), k-tile pipelined with exp 2 deep:
    S^T[k,q] = K^T.T @ Q^T   (psum, 2 heads x 512)
    E = exp(S^T) -> SBUF bf16 (ScalarE; the kernel is exp-bound: the Act
      engine runs ~131072 rows at ~1ns/row and everything else hides in
      its shadow)
    per head h, q-tile qt (128 q): psum[qt] += E[:, h, qt-slice].T @ V'
      -> [O^T | denom] with out = q on partitions, moving dim 65 (the key
      trick: half the PE rows of the feature-major orientation). Four
      65-col accumulator slots share one psum bank; the has_written
      zero-region is bracketed by a single start/stop pair per bank.
  normalize (DVE): recip of denom col, tensor_scalar per slot -> bf16.
  transpose back to feature-major (PE, bf16 identity), copy to SBUF (DVE
  2x mode), then the row-parallel output projection as in v1.
  Q/KV projections and the output projection are interleaved into the
  act-engine shadow of the attention phases.

x and the q/k/v/o weights stream in as bf16; projections accumulate in f32
PSUM, scores run f32r (full-rate fp32 PE mode), E/V in bf16, output partials
return as bf16 and are summed in f32 on the host.
"""

import numpy as np
from contextlib import ExitStack

import concourse.bass as bass
import concourse.mybir as mybir
import concourse.tile as tile
from concourse.bass import ds
from concourse import bass_utils

F32 = mybir.dt.float32
F32R = mybir.dt.float32r
BF16 = mybir.dt.bfloat16

DIM = 1024
N_HEADS = 16
N_KV_HEADS = 4
HD = 64
FH = 256                   # q features per core (4 heads x 64)
KV = 128                   # [K | V] projected feature width per core
D_TILES = DIM // 128       # 8
SEQ = 2048
BSZ = 2
N_CORES = 8


def build_attention_core(nc, S=SEQ, use_f32r=True, n_iters=1, exp_bufs=7,
                         opts=None):
    """Emit the per-core kernel. S = sequence length (multiple of 512)."""
    o = {
        "zero_ot": False,    # zeroing dummy matmul before each ot round
        "act_tail": True,    # act engine helps with tail drains
        "gates": True,       # dep-gated bulk DMA ordering
        "kt2proj": True,     # kt2 via matmul for chunks 2-3
        "ts_norm": True,     # tensor_scalar normalize (else recip+mul)
    }
    o.update(opts or {})
    QCH = 512                  # q-chunk width (psum bank = 512 f32)
    S_TILES = S // 128         # k tiles
    N_QC = S // QCH            # q chunks
    NPH = 2 * N_QC             # phases: (qc, ft)
    MDT = F32R if use_f32r else F32

    xT = nc.declare_dram_parameter("xT", [DIM, S], BF16, isOutput=False)
    identd = nc.declare_dram_parameter("ident", [128, 128], MDT, isOutput=False)
    identbd = nc.declare_dram_parameter("identb", [128, 128], BF16, isOutput=False)
    identswd = nc.declare_dram_parameter("identsw", [128, 128], MDT, isOutput=False)
    wq = nc.declare_dram_parameter("wq", [DIM, FH], BF16, isOutput=False)
    wkv = nc.declare_dram_parameter("wkv", [DIM, KV], BF16, isOutput=False)
    wo = nc.declare_dram_parameter("wo", [FH, DIM], BF16, isOutput=False)
    out = nc.declare_dram_parameter("out", [S, DIM], BF16, isOutput=True)

    with tile.TileContext(nc) as tc:
     for _it in range(n_iters):
      with ExitStack() as ctx:
        const_p = ctx.enter_context(tc.tile_pool(name="const", bufs=1))
        big_p = ctx.enter_context(tc.tile_pool(name="big", bufs=1))
        exp_p = ctx.enter_context(tc.tile_pool(name="exp", bufs=exp_bufs))
        nrm_p = ctx.enter_context(tc.tile_pool(name="nrm", bufs=6))
        stg_p = ctx.enter_context(tc.tile_pool(name="stg", bufs=6))
        ps_sc = ctx.enter_context(tc.tile_pool(name="ps_sc", bufs=2, space="PSUM"))
        ps_ot = ctx.enter_context(tc.tile_pool(name="ps_ot", bufs=2, space="PSUM"))
        ps_acc = ctx.enter_context(tc.tile_pool(name="ps_acc", bufs=2, space="PSUM"))

        # Warm the PE clock gate from a memset tile (no DMA dependency) and
        # preload the exp table set (both are real-hardware costs the cost
        # model does not charge: ~2.7us table load, 2x cold-clock ramp).
        warmb = const_p.tile([128, 128], BF16)
        nc.vector.memset(warmb[:, :], 0.0)
        warm = const_p.tile([128, 8], F32)
        nc.vector.memset(warm[0:1, 0:1], 0.0)
        nc.scalar.activation(
            warm[0:1, 1:2], warm[0:1, 0:1],
            mybir.ActivationFunctionType.Exp,
        )
        warmps = ps_sc.tile([128, 2, QCH], F32, tag="sc")
        NWARM = 12
        for w in range(NWARM):
            nc.tensor.matmul(
                warmps[:, 0, 0:128], warmb[:], warmb[:],
                start=(w == 0), stop=(w == NWARM - 1),
            )

        # ---- load inputs (chunk-0 dependencies first) --------------------
        # d-rows are consumed in (p a) order — partition p holds contraction
        # rows 8p..8p+7, which are contiguous in DRAM so the weight loads
        # collapse to one large descriptor per partition (the DMA device
        # charges ~182ns per descriptor). Any d-permutation is correct as
        # long as x, wq and wkv agree on it.
        wq_sb = big_p.tile([128, D_TILES, FH], BF16)
        wkv_sb = big_p.tile([128, D_TILES, KV], BF16)
        xt_sb = big_p.tile([128, D_TILES, S], BF16)
        nc.sync.dma_start(
            wkv_sb[:, :, :], wkv[:, :].rearrange("(p a) n -> p a n", p=128)
        )
        nc.sync.dma_start(
            wq_sb[:, :, :], wq[:, :].rearrange("(p a) n -> p a n", p=128)
        )
        nc.sync.dma_start(
            xt_sb[:, :, ds(0, 256)],
            xT[:, ds(0, 256)].rearrange("(p a) n -> p a n", p=128),
        )
        nc.sync.dma_start(
            xt_sb[:, :, ds(256, 256)],
            xT[:, ds(256, 256)].rearrange("(p a) n -> p a n", p=128),
        )
        ident = const_p.tile([128, 128], MDT)
        nc.sync.dma_start(ident[:], identd[:, :])
        identb = const_p.tile([128, 128], BF16)
        nc.sync.dma_start(identb[:], identbd[:, :])
        identsw = const_p.tile([128, 128], MDT)
        nc.sync.dma_start(identsw[:], identswd[:, :])
        wo_sb = big_p.tile([128, 2, DIM], BF16)

        # The serialized DMA device drains in ready-order, so a bulk load
        # that is ready at t=0 starves later small critical DMAs (the kt2
        # dups). Gate each bulk load on the dup DMA that must precede it: a
        # 1-element read of kt2 (RAW on the dup) written into the load's
        # destination (WAW with the load). The tile scheduler cannot hoist
        # a real data dependency.
        def load_x(c0, w, gate_col=None):
            if not o["gates"]:
                gate_col = None
            if gate_col is not None:
                nc.vector.tensor_copy(
                    xt_sb[0:1, 0, ds(c0, 1)], kvt_sb[0:1, ds(gate_col, 1)]
                )
            nc.gpsimd.dma_start(
                xt_sb[:, :, ds(c0, w)],
                xT[:, ds(c0, w)].rearrange("(p a) n -> p a n", p=128),
            )

        def load_wo(gate_col=None):
            if not o["gates"]:
                gate_col = None
            if gate_col is not None:
                nc.vector.tensor_copy(
                    wo_sb[0:1, 0, 0:1], kvt_sb[0:1, ds(gate_col, 1)]
                )
            nc.gpsimd.dma_start(
                wo_sb[:, :, :], wo[:, :].rearrange("(t p) n -> p t n", p=128)
            )

        # ---- persistent activations --------------------------------------
        kvt_sb = big_p.tile([128, S], MDT)       # [K^T | V^T] feature-major
        kt2_sb = big_p.tile([128, S], MDT)       # K^T dup at partitions 64+
        qt_sb = big_p.tile([128, 2, S], MDT)     # Q^T, per head pair
        v_aug = big_p.tile([128, S_TILES, 65], BF16)  # seq-major [V | 1]
        otT_sb = big_p.tile([128, 2, S], BF16)   # normalized O, feature-major
        nc.vector.memset(v_aug[:, :, 64:65], 1.0)

        # ---- projection / prep helpers (psum via the shared acc ring) ----
        def kvproj_piece(c0, w, dup_dma):
            # [K|V] projection for x columns [c0, c0+w); optionally emit the
            # K^T partition-dup DMA for those columns on the Pool queue.
            acc = ps_acc.tile([128, w], F32, tag="acc", name="kvacc")
            for a in range(D_TILES):
                nc.tensor.matmul(
                    acc[:],
                    wkv_sb[:, a, :],
                    xt_sb[:, a, ds(c0, w)],
                    start=(a == 0),
                    stop=(a == D_TILES - 1),
                )
            nc.vector.tensor_copy(kvt_sb[:, ds(c0, w)], acc[:])
            if dup_dma:
                ktdup_piece(c0, w)

        def ktdup_piece(c0, w):
            # K^T copied to partitions 64-127 with a single identity-weight
            # matmul (partition shift on the PE at 1 cycle/row) — no DMA
            # round trip, so the serialized DMA device carries only the
            # bulk x/w loads.
            acc = ps_acc.tile([128, w], F32, tag="acc", name="k2acc")
            nc.tensor.matmul(
                acc[:, :],
                identsw[:, :],
                kvt_sb[:, ds(c0, w)],
                start=True,
                stop=True,
            )
            nc.vector.tensor_copy(kt2_sb[64:128, ds(c0, w)], acc[64:128, :])

        def qproj_piece(qc, ft, h):
            c0 = qc * QCH + h * 256
            acc = ps_acc.tile([128, 256], F32, tag="acc", name="qacc")
            for a in range(D_TILES):
                nc.tensor.matmul(
                    acc[:],
                    wq_sb[:, a, ds(ft * 128, 128)],
                    xt_sb[:, a, ds(c0, 256)],
                    start=(a == 0),
                    stop=(a == D_TILES - 1),
                )
            nc.vector.tensor_copy(qt_sb[:, ft, ds(c0, 256)], acc[:])

        def vprep(sc, half=None):
            # V^T -> seq-major tiles, batched drain into v_aug.
            kts = range(4 * sc, 4 * sc + 4) if half is None else \
                range(4 * sc + 2 * half, 4 * sc + 2 * half + 2)
            kts = list(kts)
            trv = ps_acc.tile([128, len(kts), 64], MDT, tag="acc", name="trv")
            for i, kt in enumerate(kts):
                nc.tensor.transpose(
                    trv[:, i, :], kvt_sb[64:128, ds(kt * 128, 128)],
                    ident[64:128, 64:128],
                )
            nc.vector.tensor_copy(
                v_aug[:, ds(kts[0], len(kts)), 0:64], trv[:, :, :]
            )

        # ---- attention phase pieces --------------------------------------
        def sc_exp(qc, ft, kt):
            qsl = ds(qc * QCH, QCH)
            ksl = ds(kt * 128, 128)
            sc2 = ps_sc.tile([128, 2, QCH], F32, tag="sc")
            nc.tensor.matmul(
                sc2[:, 0, :], kvt_sb[0:64, ksl], qt_sb[0:64, ft, qsl],
                start=True, stop=True,
            )
            nc.tensor.matmul(
                sc2[:, 1, :], kt2_sb[64:128, ksl], qt_sb[64:128, ft, qsl],
                start=True, stop=True,
            )
            e2 = exp_p.tile([128, 2, QCH], BF16, tag="e")
            nc.scalar.activation(
                e2[:, :, :], sc2[:, :, :], mybir.ActivationFunctionType.Exp
            )
            return e2

        def pv(ots, e2, kt):
            # q-major PV: out[q, 0:64] += E_h[:, qslice].T @ V, col 64 = denom.
            # 4 slots share each psum bank: single start/stop pair per bank.
            for h in range(2):
                for q4 in range(4):
                    nc.tensor.matmul(
                        ots[h][:, q4, :],
                        e2[:, h, ds(q4 * 128, 128)],
                        v_aug[:, kt, :],
                        start=(kt == 0 and q4 == 0 and not o["zero_ot"]),
                        stop=(kt == S_TILES - 1 and q4 == 3),
                        skip_group_check=True,
                    )

        def zero_ot(ots):
            # zeroing dummy (0-weights) matmul covering the whole tile:
            # clears has_written for the bank and deposits exact zeros, so
            # the 4 shared accumulator slots are safe under either zero-
            # region semantics (bank-wide or range-wise).
            for h in range(2):
                nc.tensor.matmul(
                    ots[h][:, :, :], warmb[:, :], v_aug[:, 0:4, :],
                    start=True, stop=False, skip_group_check=True,
                )

        def normalize(ots):
            rc = nrm_p.tile([128, 2, 4, 1], F32, tag="rc")
            nrm = nrm_p.tile([128, 2, 4, 64], BF16, tag="nrm")
            # stage the raw psum to SBUF first: one fast copy per head frees
            # the ot-ring slot ~1us earlier at each phase boundary, so the
            # next phase's first PV matmuls are not held behind the full
            # reciprocal+scale chain.
            stg_ot = nrm_p.tile([128, 2, 4, 65], F32, tag="sot")
            for h in range(2):
                nc.vector.tensor_copy(stg_ot[:, h, :, :], ots[h][:, :, :])
            ots = (stg_ot[:, 0], stg_ot[:, 1])
            for h in range(2):
                nc.vector.reciprocal(rc[:, h, :, :], ots[h][:, :, 64:65])
                for q4 in range(4):
                    if o["ts_norm"]:
                        nc.vector.tensor_scalar_mul(
                            nrm[:, h, q4, :], ots[h][:, q4, 0:64],
                            rc[:, h, q4, :]
                        )
                    else:
                        bc = nrm_p.tile([128, 64], F32, tag="bc", name="bc")
                        nc.vector.tensor_copy(bc[:, :], rc[:, h, q4, 0:1].broadcast_to((128, 64)))
                        nc.vector.tensor_mul(
                            nrm[:, h, q4, :], ots[h][:, q4, 0:64], bc[:, :]
                        )
            return nrm

        def transposes(qc, ft, nrm, use_act=False):
            # back to feature-major: heads of the pair at partitions 0-63 /
            # 64-127 of one psum bank, then 2x-mode DVE copies to SBUF.
            trt = ps_acc.tile([128, 4, 128], BF16, tag="acc")
            for h in range(2):
                for q4 in range(4):
                    nc.tensor.transpose(
                        trt[ds(64 * h, 64), q4, :], nrm[:, h, q4, :], identb[:]
                    )
            for h in range(2):
                dst = otT_sb[ds(64 * h, 64), ft, ds(qc * QCH, QCH)]
                src = trt[ds(64 * h, 64), :, :]
                if use_act and h == 1:
                    nc.scalar.copy(dst, src)
                else:
                    nc.vector.tensor_copy(dst, src)

        def outproj_unit(qc, st, c, stg_act=False):
            row0 = qc * QCH + st * 128
            acc = ps_acc.tile([128, 512], F32, tag="acc")
            for ft in range(2):
                nc.tensor.matmul(
                    acc[:],
                    otT_sb[:, ft, ds(row0, 128)],
                    wo_sb[:, ft, ds(c * 512, 512)],
                    start=(ft == 0),
                    stop=(ft == 1),
                )
            stg = stg_p.tile([128, 512], BF16, tag="stg")
            if stg_act:
                nc.scalar.copy(stg[:], acc[:])
            else:
                nc.vector.tensor_copy(stg[:], acc[:])
            nc.sync.dma_start(out[ds(row0, 128), ds(c * 512, 512)], stg[:])

        # ---- prologue ----------------------------------------------------
        # Piecewise chunk-0 projections chase the split xt0 DMAs; the kt2
        # dups for chunks 0-1 ride the Pool queue ahead of the bulk loads.
        kvproj_piece(0, 256, dup_dma=False)
        qproj_piece(0, 0, 0)
        ktdup_piece(0, 256)
        qproj_piece(0, 0, 1)
        kvproj_piece(256, 256, dup_dma=True)
        vprep(0)
        load_x(512, 256, gate_col=255)
        load_x(768, 256, gate_col=255)
        load_x(1024, 256, gate_col=255)
        load_x(1280, 256, gate_col=255)
        load_x(1536, 256, gate_col=255)
        load_x(1792, 256, gate_col=255)
        load_wo(gate_col=255)

        # ---- interleave schedule (global kt index g = phase*16 + kt) -----
        # Projection/outproj/transpose work rides in the act-engine shadow.
        ilv = {}

        def at(g, fn):
            ilv.setdefault(g, []).append(fn)

        at(1, lambda: kvproj_piece(512, 256, dup_dma=True))
        at(2, lambda: kvproj_piece(768, 256, dup_dma=True))
        at(3, lambda: vprep(1))
        at(4, lambda: qproj_piece(0, 1, 0))
        at(5, lambda: kvproj_piece(1024, 256, dup_dma=True))
        at(6, lambda: qproj_piece(0, 1, 1))
        at(7, lambda: (kvproj_piece(1280, 256, dup_dma=True),
                       vprep(2, half=0)))
        at(8, lambda: vprep(2, half=1))
        at(9, lambda: kvproj_piece(1536, 256, dup_dma=True))
        at(10, lambda: kvproj_piece(1792, 256, dup_dma=True))
        at(11, lambda: vprep(3, half=0))
        at(12, lambda: vprep(3, half=1))
        at(19, lambda: qproj_piece(1, 0, 0))
        at(21, lambda: qproj_piece(1, 0, 1))
        at(26, lambda: qproj_piece(1, 1, 0))
        at(28, lambda: qproj_piece(1, 1, 1))
        for i in range(2, NPH):
            qc_i, ft_i = divmod(i, 2)
            if i + 2 < NPH:
                qn, fn_ = divmod(i + 2, 2)
                at(i * 16 + 2,
                   (lambda a, b: lambda: qproj_piece(a, b, 0))(qn, fn_))
                at(i * 16 + 13,
                   (lambda a, b: lambda: qproj_piece(a, b, 1))(qn, fn_))
            # outproj of q-chunk qc_i - 1 spans both phases of q-chunk qc_i
            if ft_i == 0:
                for u in range(5):
                    st, c = divmod(u, 2)
                    at(i * 16 + 4 + u,
                       (lambda a, b, cc: lambda: outproj_unit(a, b, cc))(
                           qc_i - 1, st, c))
            elif i >= 3:
                for u in range(5, 8):
                    st, c = divmod(u, 2)
                    at(i * 16 + 4 + (u - 5),
                       (lambda a, b, cc: lambda: outproj_unit(a, b, cc))(
                           qc_i - 1, st, c))

        def sc_exp_split(qc, ft, kt):
            # warm-start variant: score matmuls and exp in 256-col halves so
            # the act engine starts on the first half-drained q chunk instead
            # of waiting for the full 512-col qt drain.
            ksl = ds(kt * 128, 128)
            sc2 = ps_sc.tile([128, 2, QCH], F32, tag="sc")
            e2 = exp_p.tile([128, 2, QCH], BF16, tag="e")
            for half in range(2):
                qsl = ds(qc * QCH + half * 256, 256)
                csl = ds(half * 256, 256)
                nc.tensor.matmul(
                    sc2[:, 0, csl], kvt_sb[0:64, ksl], qt_sb[0:64, ft, qsl],
                    start=True, stop=True,
                )
                nc.tensor.matmul(
                    sc2[:, 1, csl], kt2_sb[64:128, ksl],
                    qt_sb[64:128, ft, qsl],
                    start=True, stop=True,
                )
                nc.scalar.activation(
                    e2[:, :, csl], sc2[:, :, csl],
                    mybir.ActivationFunctionType.Exp,
                )
            return e2

        # ---- main flat kt stream -----------------------------------------
        e2q = {}
        e2q[0] = sc_exp(0, 0, 0)
        e2q[1] = sc_exp(0, 0, 1)
        ots = None
        pend_tr = None  # (qc, ft, nrm) awaiting transpose
        G = NPH * S_TILES
        for g in range(G):
            i, kt = divmod(g, S_TILES)
            qc_i, ft_i = divmod(i, 2)
            if kt == 0:
                ots = (
                    ps_ot.tile([128, 4, 65], F32, tag="ot", name="ota"),
                    ps_ot.tile([128, 4, 65], F32, tag="ot", name="otb"),
                )
                if o["zero_ot"]:
                    zero_ot(ots)
            # Interleaved producers MUST be emitted before the sc/pv
            # consumers of the same g: the tile framework only sees
            # dependencies on writes emitted earlier in the stream.
            for fn in ilv.get(g, []):
                fn()
            pv(ots, e2q.pop(g), kt)
            if g + 2 < G:
                i2, kt2 = divmod(g + 2, S_TILES)
                qc2, ft2 = divmod(i2, 2)
                e2q[g + 2] = sc_exp(qc2, ft2, kt2)
            if kt == 1 and pend_tr is not None:
                transposes(*pend_tr)
                pend_tr = None
            if kt == S_TILES - 1 and g != G - 1:
                nrm = normalize(ots)
                pend_tr = (qc_i, ft_i, nrm)

        # ---- tail (fine-grained, act engine helps with drains) -----------
        # last phase: per-q-tile chains emitted one stage apart so
        # normalize/transpose/outproj/DMA pipeline across DVE/ACT/PE.
        rc7 = nrm_p.tile([128, 2, 4, 1], F32, tag="rc")
        nrm7 = nrm_p.tile([128, 2, 4, 64], BF16, tag="nrm")
        # trt7 borrows an sc-ring slot (free once the last exp has run) so it
        # does not pin the 2-slot acc ring, which the 8 outproj accs cycle.
        trt7 = ps_sc.tile([128, 4, 128], BF16, tag="sc")

        def tail_norm(q4):
            # reciprocal on DVE; the normalize multiply runs on the (now
            # idle) act engine as a scaled Copy so DVE is not the tail
            # bottleneck.
            for h in range(2):
                nc.vector.reciprocal(rc7[:, h, q4, :], ots[h][:, q4, 64:65])
                if o["act_tail"]:
                    nc.scalar.activation(
                        nrm7[:, h, q4, :], ots[h][:, q4, 0:64],
                        mybir.ActivationFunctionType.Copy,
                        scale=rc7[:, h, q4, :],
                    )
                else:
                    nc.vector.tensor_scalar_mul(
                        nrm7[:, h, q4, :], ots[h][:, q4, 0:64],
                        rc7[:, h, q4, :]
                    )

        def tail_tr(q4):
            for h in range(2):
                nc.tensor.transpose(
                    trt7[ds(64 * h, 64), q4, :], nrm7[:, h, q4, :], identb[:]
                )
                dst = otT_sb[ds(64 * h, 64), 1,
                             ds((N_QC - 1) * QCH + q4 * 128, 128)]
                nc.vector.tensor_copy(dst, trt7[ds(64 * h, 64), q4, :])

        sa = o["act_tail"]
        for q4 in range(4):
            for h in range(2):
                nc.vector.reciprocal(rc7[:, h, q4, :], ots[h][:, q4, 64:65])
        for q4 in range(4):
            for h in range(2):
                if o["act_tail"]:
                    nc.scalar.activation(
                        nrm7[:, h, q4, :], ots[h][:, q4, 0:64],
                        mybir.ActivationFunctionType.Copy,
                        scale=rc7[:, h, q4, :],
                    )
                else:
                    nc.vector.tensor_scalar_mul(
                        nrm7[:, h, q4, :], ots[h][:, q4, 0:64],
                        rc7[:, h, q4, :]
                    )
        for q4 in range(4):
            for h in range(2):
                nc.tensor.transpose(
                    trt7[ds(64 * h, 64), q4, :], nrm7[:, h, q4, :], identb[:]
                )
        for q4 in range(4):
            for h in range(2):
                nc.vector.tensor_copy(
                    otT_sb[ds(64 * h, 64), 1,
                           ds((N_QC - 1) * QCH + q4 * 128, 128)],
                    trt7[ds(64 * h, 64), q4, :],
                )
        for st in range(4):
            outproj_unit(N_QC - 1, st, 0)
            outproj_unit(N_QC - 1, st, 1, stg_act=sa)

    return nc


# The neuronx compiler in this environment accepts only ONE sync-wait command
# per instruction; Tile emits instructions with several. Waiting is monotone,
# so hoisting all but the last wait onto same-engine NoOps is equivalent.
_wsctr = [0]


def split_multi_waits(nc):
    n_split = 0
    for f in nc.m.functions:
        for bb in f.blocks:
            insts = bb.instructions
            if not any(
                i.sync_info is not None and len(i.sync_info.on_wait) > 1
                for i in insts
            ):
                continue
            new = []
            for i in insts:
                si = i.sync_info
                if si is not None and len(si.on_wait) > 1:
                    waits = list(si.on_wait)
                    for w in waits[:-1]:
                        _wsctr[0] += 1
                        nop = mybir.InstNoOp(name=f"wsplit_{_wsctr[0]}", ins=[], outs=[])
                        nop.engine = i.engine
                        nop.sync_info = mybir.SyncInfo(on_wait=[w], on_update=[])
                        new.append(nop)
                    i.sync_info = mybir.SyncInfo(
                        on_wait=[waits[-1]], on_update=list(si.on_update)
                    )
                    n_split += 1
                new.append(i)
            bb.instructions = new
    return n_split


def build(use_f32r=True):
    nc = bass.Bass(target_bir_lowering=False)
    build_attention_core(nc, SEQ, use_f32r=use_f32r)
    split_multi_waits(nc)
    return nc


def shard_inputs(x, wq, wk, wv, wo):
    """Full inputs -> per-core in_maps. Core c = (b = c//4, g = c%4)."""
    x = np.asarray(x, np.float32)
    wq = np.asarray(wq, np.float32)
    wk = np.asarray(wk, np.float32)
    wv = np.asarray(wv, np.float32)
    wo = np.asarray(wo, np.float32)
    ident = np.eye(128, dtype=np.float32)
    import ml_dtypes
    bf16 = ml_dtypes.bfloat16
    identb = np.eye(128, dtype=np.float32).astype(bf16)
    identsw = np.roll(np.eye(128, dtype=np.float32), 64, axis=1)
    xTs = [np.ascontiguousarray(x[b].T).astype(bf16) for b in range(BSZ)]
    in_maps = []
    for c in range(N_CORES):
        b, g = c // 4, c % 4
        # fold the 1/sqrt(head_dim) score scaling into wq
        wq_g = (np.ascontiguousarray(wq[:, g * FH:(g + 1) * FH]) * (1.0 / np.sqrt(HD))).astype(bf16)
        wkv_g = np.ascontiguousarray(
            np.concatenate(
                [wk[:, g * HD:(g + 1) * HD], wv[:, g * HD:(g + 1) * HD]], axis=1
            )
        ).astype(bf16)
        wo_g = np.ascontiguousarray(wo[g * FH:(g + 1) * FH, :]).astype(bf16)
        in_maps.append(
            {"xT": xTs[b], "wq": wq_g, "wkv": wkv_g, "wo": wo_g,
             "ident": ident, "identb": identb, "identsw": identsw}
        )
    return in_maps


def unshard_output(results):
    """Sum the 4 row-parallel partial outputs per batch."""
    out = np.zeros((BSZ, SEQ, DIM), np.float32)
    for c in range(N_CORES):
        out[c // 4] += np.asarray(results[c]["out"], np.float32)
    return out


_cache = {}


def kernel(x, wq, wk, wv, wo):
    if "nc" not in _cache:
        _cache["nc"] = build()
    nc = _cache["nc"]
    in_maps = shard_inputs(x, wq, wk, wv, wo)
    try:
        res = bass_utils.run_bass_kernel_spmd(
            nc, in_maps, core_ids=list(range(N_CORES))
        )
    except ModuleNotFoundError:
        # BASS_TRACE under an axon client without the NTFF hook module;
        # rerun untraced.
        import os

        os.environ["BASS_NEVER_TRACE"] = "1"
        res = bass_utils.run_bass_kernel_spmd(
            nc, in_maps, core_ids=list(range(N_CORES))
        )
    return unshard_output(res.results)


# revision 63
# speedup vs baseline: 1.0104x; 1.0104x over previous
"""Distributed attention kernel for Trainium2 (8 NeuronCores).

Problem: non-causal multi-head attention with GQA (16 q heads, 4 kv heads,
head_dim 64, dim 1024, batch 2, seqlen 2048), fp32.

Sharding (per the batch+head hint): core c in 0..7 handles batch b = c//4
and kv-head-group g = c%4 (q heads 4g..4g+3, kv head g). Each core holds the
full sequence, so softmax needs no communication. The output projection is
row-parallel: core (b, g) computes the partial product
O_g @ wo[256g:256(g+1), :] and the host sums the 4 partials per batch
(the gather/unshard step).

Per-core dataflow (v2 — exp-bound pipeline, PV in q-major orientation):
  xT = x[b].T                               (1024, S) fed from host
  QT = wq_g.T @ xT                          (256, S)  [head pair ft: rows
                                              0-63 = head 2ft, 64-127 = 2ft+1]
  KVT = [wk_g | wv_g].T @ xT                (128, S)  [K^T | V^T stacked]
  K^T duplicated to partitions 64-127 (swap-permutation identity matmul,
  1 cycle/row) so the two heads of a pair run as row-group-tiled matmuls
  on the PE array.
  V transposed (PE) to seq-major and packed as V' = [V | 1] (65 cols): the
  ones column makes the PV matmul produce the softmax denominator in place.
  Per (head pair ft, q-chunk 512), k-tile pipelined with exp 2 deep:
    S^T[k,q] = K^T.T @ Q^T   (psum, 2 heads x 512)
    E = exp(S^T) -> SBUF bf16 (ScalarE; the kernel is exp-bound: the Act
      engine runs ~131072 rows at ~1ns/row and everything else hides in
      its shadow)
    per head h, q-tile qt (128 q): psum[qt] += E[:, h, qt-slice].T @ V'
      -> [O^T | denom] with out = q on partitions, moving dim 65 (the key
      trick: half the PE rows of the feature-major orientation). Four
      65-col accumulator slots share one psum bank; the has_written
      zero-region is bracketed by a single start/stop pair per bank.
  normalize (DVE): recip of denom col, tensor_scalar per slot -> bf16.
  transpose back to feature-major (PE, bf16 identity), copy to SBUF (DVE
  2x mode), then the row-parallel output projection as in v1.
  Q/KV projections and the output projection are interleaved into the
  act-engine shadow of the attention phases.

x and the q/k/v/o weights stream in as bf16; projections accumulate in f32
PSUM, scores run f32r (full-rate fp32 PE mode), E/V in bf16, output partials
return as bf16 and are summed in f32 on the host.
"""

import numpy as np
from contextlib import ExitStack

import concourse.bass as bass
import concourse.mybir as mybir
import concourse.tile as tile
from concourse.bass import ds
from concourse import bass_utils

F32 = mybir.dt.float32
F32R = mybir.dt.float32r
BF16 = mybir.dt.bfloat16

DIM = 1024
N_HEADS = 16
N_KV_HEADS = 4
HD = 64
FH = 256                   # q features per core (4 heads x 64)
KV = 128                   # [K | V] projected feature width per core
D_TILES = DIM // 128       # 8
SEQ = 2048
BSZ = 2
N_CORES = 8


def build_attention_core(nc, S=SEQ, use_f32r=True, n_iters=1, exp_bufs=7,
                         opts=None):
    """Emit the per-core kernel. S = sequence length (multiple of 512)."""
    o = {
        "zero_ot": False,    # zeroing dummy matmul before each ot round
        "act_tail": True,    # act engine helps with tail drains
        "gates": True,       # dep-gated bulk DMA ordering
        "kt2proj": True,     # kt2 via matmul for chunks 2-3
        "ts_norm": True,     # tensor_scalar normalize (else recip+mul)
    }
    o.update(opts or {})
    QCH = 512                  # q-chunk width (psum bank = 512 f32)
    S_TILES = S // 128         # k tiles
    N_QC = S // QCH            # q chunks
    NPH = 2 * N_QC             # phases: (qc, ft)
    MDT = F32R if use_f32r else F32

    xT = nc.declare_dram_parameter("xT", [DIM, S], BF16, isOutput=False)
    identd = nc.declare_dram_parameter("ident", [128, 128], MDT, isOutput=False)
    identbd = nc.declare_dram_parameter("identb", [128, 128], BF16, isOutput=False)
    identswd = nc.declare_dram_parameter("identsw", [128, 128], MDT, isOutput=False)
    wq = nc.declare_dram_parameter("wq", [DIM, FH], BF16, isOutput=False)
    wkv = nc.declare_dram_parameter("wkv", [DIM, KV], BF16, isOutput=False)
    wo = nc.declare_dram_parameter("wo", [FH, DIM], BF16, isOutput=False)
    out = nc.declare_dram_parameter("out", [S, DIM], BF16, isOutput=True)

    with tile.TileContext(nc) as tc:
     for _it in range(n_iters):
      with ExitStack() as ctx:
        const_p = ctx.enter_context(tc.tile_pool(name="const", bufs=1))
        big_p = ctx.enter_context(tc.tile_pool(name="big", bufs=1))
        exp_p = ctx.enter_context(tc.tile_pool(name="exp", bufs=exp_bufs))
        nrm_p = ctx.enter_context(tc.tile_pool(name="nrm", bufs=6))
        stg_p = ctx.enter_context(tc.tile_pool(name="stg", bufs=6))
        ps_sc = ctx.enter_context(tc.tile_pool(name="ps_sc", bufs=2, space="PSUM"))
        ps_ot = ctx.enter_context(tc.tile_pool(name="ps_ot", bufs=2, space="PSUM"))
        ps_acc = ctx.enter_context(tc.tile_pool(name="ps_acc", bufs=2, space="PSUM"))

        # Warm the PE clock gate from a memset tile (no DMA dependency) and
        # preload the exp table set (both are real-hardware costs the cost
        # model does not charge: ~2.7us table load, 2x cold-clock ramp).
        warmb = const_p.tile([128, 128], BF16)
        nc.vector.memset(warmb[:, :], 0.0)
        warm = const_p.tile([128, 8], F32)
        nc.vector.memset(warm[0:1, 0:1], 0.0)
        nc.scalar.activation(
            warm[0:1, 1:2], warm[0:1, 0:1],
            mybir.ActivationFunctionType.Exp,
        )
        warmps = ps_sc.tile([128, 2, QCH], F32, tag="sc")
        NWARM = 12
        for w in range(NWARM):
            nc.tensor.matmul(
                warmps[:, 0, 0:128], warmb[:], warmb[:],
                start=(w == 0), stop=(w == NWARM - 1),
            )

        # ---- load inputs (chunk-0 dependencies first) --------------------
        # d-rows are consumed in (p a) order — partition p holds contraction
        # rows 8p..8p+7, which are contiguous in DRAM so the weight loads
        # collapse to one large descriptor per partition (the DMA device
        # charges ~182ns per descriptor). Any d-permutation is correct as
        # long as x, wq and wkv agree on it.
        wq_sb = big_p.tile([128, D_TILES, FH], BF16)
        wkv_sb = big_p.tile([128, D_TILES, KV], BF16)
        xt_sb = big_p.tile([128, D_TILES, S], BF16)
        nc.sync.dma_start(
            wkv_sb[:, :, :], wkv[:, :].rearrange("(p a) n -> p a n", p=128)
        )
        nc.sync.dma_start(
            wq_sb[:, :, :], wq[:, :].rearrange("(p a) n -> p a n", p=128)
        )
        nc.sync.dma_start(
            xt_sb[:, :, ds(0, 256)],
            xT[:, ds(0, 256)].rearrange("(p a) n -> p a n", p=128),
        )
        nc.sync.dma_start(
            xt_sb[:, :, ds(256, 256)],
            xT[:, ds(256, 256)].rearrange("(p a) n -> p a n", p=128),
        )
        ident = const_p.tile([128, 128], MDT)
        nc.sync.dma_start(ident[:], identd[:, :])
        identb = const_p.tile([128, 128], BF16)
        nc.sync.dma_start(identb[:], identbd[:, :])
        identsw = const_p.tile([128, 128], MDT)
        nc.sync.dma_start(identsw[:], identswd[:, :])
        wo_sb = big_p.tile([128, 2, DIM], BF16)

        # The serialized DMA device drains in ready-order, so a bulk load
        # that is ready at t=0 starves later small critical DMAs (the kt2
        # dups). Gate each bulk load on the dup DMA that must precede it: a
        # 1-element read of kt2 (RAW on the dup) written into the load's
        # destination (WAW with the load). The tile scheduler cannot hoist
        # a real data dependency.
        def load_x(c0, w, gate_col=None):
            if not o["gates"]:
                gate_col = None
            if gate_col is not None:
                nc.vector.tensor_copy(
                    xt_sb[0:1, 0, ds(c0, 1)], kvt_sb[0:1, ds(gate_col, 1)]
                )
            nc.gpsimd.dma_start(
                xt_sb[:, :, ds(c0, w)],
                xT[:, ds(c0, w)].rearrange("(p a) n -> p a n", p=128),
            )

        def load_wo(gate_col=None):
            if not o["gates"]:
                gate_col = None
            if gate_col is not None:
                nc.vector.tensor_copy(
                    wo_sb[0:1, 0, 0:1], kvt_sb[0:1, ds(gate_col, 1)]
                )
            nc.gpsimd.dma_start(
                wo_sb[:, :, :], wo[:, :].rearrange("(t p) n -> p t n", p=128)
            )

        # ---- persistent activations --------------------------------------
        kvt_sb = big_p.tile([128, S], MDT)       # [K^T | V^T] feature-major
        kt2_sb = big_p.tile([128, S], MDT)       # K^T dup at partitions 64+
        qt_sb = big_p.tile([128, 2, S], MDT)     # Q^T, per head pair
        v_aug = big_p.tile([128, S_TILES, 65], BF16)  # seq-major [V | 1]
        otT_sb = big_p.tile([128, 2, S], BF16)   # normalized O, feature-major
        nc.vector.memset(v_aug[:, :, 64:65], 1.0)

        # ---- projection / prep helpers (psum via the shared acc ring) ----
        def kvproj_piece(c0, w, dup_dma):
            # [K|V] projection for x columns [c0, c0+w); optionally emit the
            # K^T partition-dup DMA for those columns on the Pool queue.
            acc = ps_acc.tile([128, w], F32, tag="acc", name="kvacc")
            for a in range(D_TILES):
                nc.tensor.matmul(
                    acc[:],
                    wkv_sb[:, a, :],
                    xt_sb[:, a, ds(c0, w)],
                    start=(a == 0),
                    stop=(a == D_TILES - 1),
                )
            nc.vector.tensor_copy(kvt_sb[:, ds(c0, w)], acc[:])
            if dup_dma:
                ktdup_piece(c0, w)

        def ktdup_piece(c0, w):
            # K^T copied to partitions 64-127 with a single identity-weight
            # matmul (partition shift on the PE at 1 cycle/row) — no DMA
            # round trip, so the serialized DMA device carries only the
            # bulk x/w loads.
            acc = ps_acc.tile([128, w], F32, tag="acc", name="k2acc")
            nc.tensor.matmul(
                acc[:, :],
                identsw[:, :],
                kvt_sb[:, ds(c0, w)],
                start=True,
                stop=True,
            )
            nc.vector.tensor_copy(kt2_sb[64:128, ds(c0, w)], acc[64:128, :])

        def qproj_piece(qc, ft, h):
            c0 = qc * QCH + h * 256
            acc = ps_acc.tile([128, 256], F32, tag="acc", name="qacc")
            for a in range(D_TILES):
                nc.tensor.matmul(
                    acc[:],
                    wq_sb[:, a, ds(ft * 128, 128)],
                    xt_sb[:, a, ds(c0, 256)],
                    start=(a == 0),
                    stop=(a == D_TILES - 1),
                )
            nc.vector.tensor_copy(qt_sb[:, ft, ds(c0, 256)], acc[:])

        def vprep(sc, half=None):
            # V^T -> seq-major tiles, batched drain into v_aug.
            kts = range(4 * sc, 4 * sc + 4) if half is None else \
                range(4 * sc + 2 * half, 4 * sc + 2 * half + 2)
            kts = list(kts)
            trv = ps_acc.tile([128, len(kts), 64], MDT, tag="acc", name="trv")
            for i, kt in enumerate(kts):
                nc.tensor.transpose(
                    trv[:, i, :], kvt_sb[64:128, ds(kt * 128, 128)],
                    ident[64:128, 64:128],
                )
            nc.vector.tensor_copy(
                v_aug[:, ds(kts[0], len(kts)), 0:64], trv[:, :, :]
            )

        # ---- attention phase pieces --------------------------------------
        def sc_exp(qc, ft, kt):
            qsl = ds(qc * QCH, QCH)
            ksl = ds(kt * 128, 128)
            sc2 = ps_sc.tile([128, 2, QCH], F32, tag="sc")
            nc.tensor.matmul(
                sc2[:, 0, :], kvt_sb[0:64, ksl], qt_sb[0:64, ft, qsl],
                start=True, stop=True,
            )
            nc.tensor.matmul(
                sc2[:, 1, :], kt2_sb[64:128, ksl], qt_sb[64:128, ft, qsl],
                start=True, stop=True,
            )
            e2 = exp_p.tile([128, 2, QCH], BF16, tag="e")
            nc.scalar.activation(
                e2[:, :, :], sc2[:, :, :], mybir.ActivationFunctionType.Exp
            )
            return e2

        def pv(ots, e2, kt):
            # q-major PV: out[q, 0:64] += E_h[:, qslice].T @ V, col 64 = denom.
            # 4 slots share each psum bank: single start/stop pair per bank.
            for h in range(2):
                for q4 in range(4):
                    nc.tensor.matmul(
                        ots[h][:, q4, :],
                        e2[:, h, ds(q4 * 128, 128)],
                        v_aug[:, kt, :],
                        start=(kt == 0 and q4 == 0 and not o["zero_ot"]),
                        stop=(kt == S_TILES - 1 and q4 == 3),
                        skip_group_check=True,
                    )

        def zero_ot(ots):
            # zeroing dummy (0-weights) matmul covering the whole tile:
            # clears has_written for the bank and deposits exact zeros, so
            # the 4 shared accumulator slots are safe under either zero-
            # region semantics (bank-wide or range-wise).
            for h in range(2):
                nc.tensor.matmul(
                    ots[h][:, :, :], warmb[:, :], v_aug[:, 0:4, :],
                    start=True, stop=False, skip_group_check=True,
                )

        def normalize(ots):
            rc = nrm_p.tile([128, 2, 4, 1], F32, tag="rc")
            nrm = nrm_p.tile([128, 2, 4, 64], BF16, tag="nrm")
            # stage the raw psum to SBUF first: one fast copy per head frees
            # the ot-ring slot ~1us earlier at each phase boundary, so the
            # next phase's first PV matmuls are not held behind the full
            # reciprocal+scale chain.
            stg_ot = nrm_p.tile([128, 2, 4, 65], F32, tag="sot")
            for h in range(2):
                nc.vector.tensor_copy(stg_ot[:, h, :, :], ots[h][:, :, :])
            ots = (stg_ot[:, 0], stg_ot[:, 1])
            for h in range(2):
                nc.vector.reciprocal(rc[:, h, :, :], ots[h][:, :, 64:65])
                for q4 in range(4):
                    if o["ts_norm"]:
                        nc.vector.tensor_scalar_mul(
                            nrm[:, h, q4, :], ots[h][:, q4, 0:64],
                            rc[:, h, q4, :]
                        )
                    else:
                        bc = nrm_p.tile([128, 64], F32, tag="bc", name="bc")
                        nc.vector.tensor_copy(bc[:, :], rc[:, h, q4, 0:1].broadcast_to((128, 64)))
                        nc.vector.tensor_mul(
                            nrm[:, h, q4, :], ots[h][:, q4, 0:64], bc[:, :]
                        )
            return nrm

        def transposes(qc, ft, nrm, use_act=False):
            # back to feature-major: heads of the pair at partitions 0-63 /
            # 64-127 of one psum bank, then 2x-mode DVE copies to SBUF.
            trt = ps_acc.tile([128, 4, 128], BF16, tag="acc")
            for h in range(2):
                for q4 in range(4):
                    nc.tensor.transpose(
                        trt[ds(64 * h, 64), q4, :], nrm[:, h, q4, :], identb[:]
                    )
            for h in range(2):
                dst = otT_sb[ds(64 * h, 64), ft, ds(qc * QCH, QCH)]
                src = trt[ds(64 * h, 64), :, :]
                if use_act and h == 1:
                    nc.scalar.copy(dst, src)
                else:
                    nc.vector.tensor_copy(dst, src)

        def outproj_unit(qc, st, c, stg_act=False):
            row0 = qc * QCH + st * 128
            acc = ps_acc.tile([128, 512], F32, tag="acc")
            for ft in range(2):
                nc.tensor.matmul(
                    acc[:],
                    otT_sb[:, ft, ds(row0, 128)],
                    wo_sb[:, ft, ds(c * 512, 512)],
                    start=(ft == 0),
                    stop=(ft == 1),
                )
            stg = stg_p.tile([128, 512], BF16, tag="stg")
            if stg_act:
                nc.scalar.copy(stg[:], acc[:])
            else:
                nc.vector.tensor_copy(stg[:], acc[:])
            nc.sync.dma_start(out[ds(row0, 128), ds(c * 512, 512)], stg[:])

        # ---- prologue ----------------------------------------------------
        # Piecewise chunk-0 projections chase the split xt0 DMAs; the kt2
        # dups for chunks 0-1 ride the Pool queue ahead of the bulk loads.
        kvproj_piece(0, 256, dup_dma=False)
        qproj_piece(0, 0, 0)
        ktdup_piece(0, 256)
        qproj_piece(0, 0, 1)
        kvproj_piece(256, 256, dup_dma=True)
        vprep(0)
        load_x(512, 256, gate_col=255)
        load_x(768, 256, gate_col=255)
        load_x(1024, 256, gate_col=255)
        load_x(1280, 256, gate_col=255)
        load_x(1536, 256, gate_col=255)
        load_x(1792, 256, gate_col=255)
        load_wo(gate_col=255)

        # ---- interleave schedule (global kt index g = phase*16 + kt) -----
        # Projection/outproj/transpose work rides in the act-engine shadow.
        ilv = {}

        def at(g, fn):
            ilv.setdefault(g, []).append(fn)

        at(1, lambda: kvproj_piece(512, 256, dup_dma=True))
        at(2, lambda: kvproj_piece(768, 256, dup_dma=True))
        at(3, lambda: vprep(1))
        at(4, lambda: qproj_piece(0, 1, 0))
        at(5, lambda: kvproj_piece(1024, 256, dup_dma=True))
        at(6, lambda: qproj_piece(0, 1, 1))
        at(7, lambda: (kvproj_piece(1280, 256, dup_dma=True),
                       vprep(2, half=0)))
        at(8, lambda: vprep(2, half=1))
        at(9, lambda: kvproj_piece(1536, 256, dup_dma=True))
        at(10, lambda: kvproj_piece(1792, 256, dup_dma=True))
        at(11, lambda: vprep(3, half=0))
        at(12, lambda: vprep(3, half=1))
        at(19, lambda: qproj_piece(1, 0, 0))
        at(21, lambda: qproj_piece(1, 0, 1))
        at(26, lambda: qproj_piece(1, 1, 0))
        at(28, lambda: qproj_piece(1, 1, 1))
        for i in range(2, NPH):
            qc_i, ft_i = divmod(i, 2)
            if i + 2 < NPH:
                qn, fn_ = divmod(i + 2, 2)
                at(i * 16 + 2,
                   (lambda a, b: lambda: qproj_piece(a, b, 0))(qn, fn_))
                at(i * 16 + 13,
                   (lambda a, b: lambda: qproj_piece(a, b, 1))(qn, fn_))
            # outproj of q-chunk qc_i - 1 spans both phases of q-chunk qc_i
            if ft_i == 0:
                for u in range(5):
                    st, c = divmod(u, 2)
                    at(i * 16 + 4 + u,
                       (lambda a, b, cc: lambda: outproj_unit(a, b, cc))(
                           qc_i - 1, st, c))
            elif i >= 3:
                for u in range(5, 8):
                    st, c = divmod(u, 2)
                    at(i * 16 + 4 + (u - 5),
                       (lambda a, b, cc: lambda: outproj_unit(a, b, cc))(
                           qc_i - 1, st, c))

        def sc_exp_split(qc, ft, kt):
            # warm-start variant: score matmuls and exp in 256-col halves so
            # the act engine starts on the first half-drained q chunk instead
            # of waiting for the full 512-col qt drain.
            ksl = ds(kt * 128, 128)
            sc2 = ps_sc.tile([128, 2, QCH], F32, tag="sc")
            e2 = exp_p.tile([128, 2, QCH], BF16, tag="e")
            for half in range(2):
                qsl = ds(qc * QCH + half * 256, 256)
                csl = ds(half * 256, 256)
                nc.tensor.matmul(
                    sc2[:, 0, csl], kvt_sb[0:64, ksl], qt_sb[0:64, ft, qsl],
                    start=True, stop=True,
                )
                nc.tensor.matmul(
                    sc2[:, 1, csl], kt2_sb[64:128, ksl],
                    qt_sb[64:128, ft, qsl],
                    start=True, stop=True,
                )
                nc.scalar.activation(
                    e2[:, :, csl], sc2[:, :, csl],
                    mybir.ActivationFunctionType.Exp,
                )
            return e2

        # ---- main flat kt stream -----------------------------------------
        e2q = {}
        e2q[0] = sc_exp(0, 0, 0)
        e2q[1] = sc_exp(0, 0, 1)
        ots = None
        pend_tr = None  # (qc, ft, nrm) awaiting transpose
        G = NPH * S_TILES
        for g in range(G):
            i, kt = divmod(g, S_TILES)
            qc_i, ft_i = divmod(i, 2)
            if kt == 0:
                ots = (
                    ps_ot.tile([128, 4, 65], F32, tag="ot", name="ota"),
                    ps_ot.tile([128, 4, 65], F32, tag="ot", name="otb"),
                )
                if o["zero_ot"]:
                    zero_ot(ots)
            # Interleaved producers MUST be emitted before the sc/pv
            # consumers of the same g: the tile framework only sees
            # dependencies on writes emitted earlier in the stream.
            for fn in ilv.get(g, []):
                fn()
            pv(ots, e2q.pop(g), kt)
            if g + 2 < G:
                i2, kt2 = divmod(g + 2, S_TILES)
                qc2, ft2 = divmod(i2, 2)
                e2q[g + 2] = sc_exp(qc2, ft2, kt2)
            if kt == 2 and pend_tr is not None:
                transposes(*pend_tr)
                pend_tr = None
            if kt == S_TILES - 1 and g != G - 1:
                nrm = normalize(ots)
                pend_tr = (qc_i, ft_i, nrm)

        # ---- tail (fine-grained, act engine helps with drains) -----------
        # last phase: per-q-tile chains emitted one stage apart so
        # normalize/transpose/outproj/DMA pipeline across DVE/ACT/PE.
        rc7 = nrm_p.tile([128, 2, 4, 1], F32, tag="rc")
        nrm7 = nrm_p.tile([128, 2, 4, 64], BF16, tag="nrm")
        # trt7 borrows an sc-ring slot (free once the last exp has run) so it
        # does not pin the 2-slot acc ring, which the 8 outproj accs cycle.
        trt7 = ps_sc.tile([128, 4, 128], BF16, tag="sc")

        def tail_norm(q4):
            # reciprocal on DVE; the normalize multiply runs on the (now
            # idle) act engine as a scaled Copy so DVE is not the tail
            # bottleneck.
            for h in range(2):
                nc.vector.reciprocal(rc7[:, h, q4, :], ots[h][:, q4, 64:65])
                if o["act_tail"]:
                    nc.scalar.activation(
                        nrm7[:, h, q4, :], ots[h][:, q4, 0:64],
                        mybir.ActivationFunctionType.Copy,
                        scale=rc7[:, h, q4, :],
                    )
                else:
                    nc.vector.tensor_scalar_mul(
                        nrm7[:, h, q4, :], ots[h][:, q4, 0:64],
                        rc7[:, h, q4, :]
                    )

        def tail_tr(q4):
            for h in range(2):
                nc.tensor.transpose(
                    trt7[ds(64 * h, 64), q4, :], nrm7[:, h, q4, :], identb[:]
                )
                dst = otT_sb[ds(64 * h, 64), 1,
                             ds((N_QC - 1) * QCH + q4 * 128, 128)]
                nc.vector.tensor_copy(dst, trt7[ds(64 * h, 64), q4, :])

        sa = o["act_tail"]
        for q4 in range(4):
            for h in range(2):
                nc.vector.reciprocal(rc7[:, h, q4, :], ots[h][:, q4, 64:65])
        for q4 in range(4):
            for h in range(2):
                if o["act_tail"]:
                    nc.scalar.activation(
                        nrm7[:, h, q4, :], ots[h][:, q4, 0:64],
                        mybir.ActivationFunctionType.Copy,
                        scale=rc7[:, h, q4, :],
                    )
                else:
                    nc.vector.tensor_scalar_mul(
                        nrm7[:, h, q4, :], ots[h][:, q4, 0:64],
                        rc7[:, h, q4, :]
                    )
        for q4 in range(4):
            for h in range(2):
                nc.tensor.transpose(
                    trt7[ds(64 * h, 64), q4, :], nrm7[:, h, q4, :], identb[:]
                )
        for q4 in range(4):
            for h in range(2):
                nc.vector.tensor_copy(
                    otT_sb[ds(64 * h, 64), 1,
                           ds((N_QC - 1) * QCH + q4 * 128, 128)],
                    trt7[ds(64 * h, 64), q4, :],
                )
        for st in range(4):
            outproj_unit(N_QC - 1, st, 0)
            outproj_unit(N_QC - 1, st, 1, stg_act=sa)

    return nc


# The neuronx compiler in this environment accepts only ONE sync-wait command
# per instruction; Tile emits instructions with several. Waiting is monotone,
# so hoisting all but the last wait onto same-engine NoOps is equivalent.
_wsctr = [0]


def split_multi_waits(nc):
    n_split = 0
    for f in nc.m.functions:
        for bb in f.blocks:
            insts = bb.instructions
            if not any(
                i.sync_info is not None and len(i.sync_info.on_wait) > 1
                for i in insts
            ):
                continue
            new = []
            for i in insts:
                si = i.sync_info
                if si is not None and len(si.on_wait) > 1:
                    waits = list(si.on_wait)
                    for w in waits[:-1]:
                        _wsctr[0] += 1
                        nop = mybir.InstNoOp(name=f"wsplit_{_wsctr[0]}", ins=[], outs=[])
                        nop.engine = i.engine
                        nop.sync_info = mybir.SyncInfo(on_wait=[w], on_update=[])
                        new.append(nop)
                    i.sync_info = mybir.SyncInfo(
                        on_wait=[waits[-1]], on_update=list(si.on_update)
                    )
                    n_split += 1
                new.append(i)
            bb.instructions = new
    return n_split


def build(use_f32r=True):
    nc = bass.Bass(target_bir_lowering=False)
    build_attention_core(nc, SEQ, use_f32r=use_f32r)
    split_multi_waits(nc)
    return nc


def shard_inputs(x, wq, wk, wv, wo):
    """Full inputs -> per-core in_maps. Core c = (b = c//4, g = c%4)."""
    x = np.asarray(x, np.float32)
    wq = np.asarray(wq, np.float32)
    wk = np.asarray(wk, np.float32)
    wv = np.asarray(wv, np.float32)
    wo = np.asarray(wo, np.float32)
    ident = np.eye(128, dtype=np.float32)
    import ml_dtypes
    bf16 = ml_dtypes.bfloat16
    identb = np.eye(128, dtype=np.float32).astype(bf16)
    identsw = np.roll(np.eye(128, dtype=np.float32), 64, axis=1)
    xTs = [np.ascontiguousarray(x[b].T).astype(bf16) for b in range(BSZ)]
    in_maps = []
    for c in range(N_CORES):
        b, g = c // 4, c % 4
        # fold the 1/sqrt(head_dim) score scaling into wq
        wq_g = (np.ascontiguousarray(wq[:, g * FH:(g + 1) * FH]) * (1.0 / np.sqrt(HD))).astype(bf16)
        wkv_g = np.ascontiguousarray(
            np.concatenate(
                [wk[:, g * HD:(g + 1) * HD], wv[:, g * HD:(g + 1) * HD]], axis=1
            )
        ).astype(bf16)
        wo_g = np.ascontiguousarray(wo[g * FH:(g + 1) * FH, :]).astype(bf16)
        in_maps.append(
            {"xT": xTs[b], "wq": wq_g, "wkv": wkv_g, "wo": wo_g,
             "ident": ident, "identb": identb, "identsw": identsw}
        )
    return in_maps


def unshard_output(results):
    """Sum the 4 row-parallel partial outputs per batch."""
    out = np.zeros((BSZ, SEQ, DIM), np.float32)
    for c in range(N_CORES):
        out[c // 4] += np.asarray(results[c]["out"], np.float32)
    return out


_cache = {}


def kernel(x, wq, wk, wv, wo):
    if "nc" not in _cache:
        _cache["nc"] = build()
    nc = _cache["nc"]
    in_maps = shard_inputs(x, wq, wk, wv, wo)
    try:
        res = bass_utils.run_bass_kernel_spmd(
            nc, in_maps, core_ids=list(range(N_CORES))
        )
    except ModuleNotFoundError:
        # BASS_TRACE under an axon client without the NTFF hook module;
        # rerun untraced.
        import os

        os.environ["BASS_NEVER_TRACE"] = "1"
        res = bass_utils.run_bass_kernel_spmd(
            nc, in_maps, core_ids=list(range(N_CORES))
        )
    return unshard_output(res.results)
